# revision 17
# baseline (speedup 1.0000x reference)
"""LSTM critic kernel for Trainium2 (8 NeuronCores, data-parallel over batch).

Reference computation (per sequence, T=256 steps, hidden H=64):
    gates = [x_t, h] @ W_lstm + b_lstm          # gate order i, j, f, o
    c' = c * sigmoid(f + 1) + sigmoid(i) * tanh(j)
    h' = tanh(c') * sigmoid(o)
    out_t = h' @ W_dec + b_dec

Device strategy (per core, batch shard of 512 split into NCH chains):
  - X input (with a constant-1 row carrying the biases) is streamed into
    SBUF in XCHUNK-step windows per chain (one DMA per window, multi
    buffered) — no per-step input DMAs, so the serialized DMA descriptor
    generators (~625-1000ns per DMA) never throttle the step cadence.
  - Per chain-step, four matmuls accumulate into one PSUM tile [128, 2*CB]:
    x-parts (K=41) fire early off the critical path (PSUM start=True);
    h-parts (K=64, rhs = previous h tile) are the recurrence head.
    Columns [0:CB] = (o', i') gates stacked on partitions, [CB:2CB] = (f', j').
    Weights pre-scaled on host so every gate activation is sigmoid(2*x):
      o' = (o + b_o)/2, i' = (i + b_i)/2, f' = (f + b_f + 1)/2, j' = j + b_j
    tanh(j) = 2*sigmoid(2j) - 1 (one cheap tensor_scalar fixup on DVE).
  - One sigmoid ACT op per chain-step over the whole PSUM tile; tanh(c') is
    the only other ACT op (same table set, no reloads). With 3 chains the
    ACT sequencer stream (2 ops per chain-step) stays busy through the
    cell-update latency of each individual chain.
  - Ops are emitted phase-grouped across chains: engine sequencers stall
    in-order on semaphore waits, so a waiting op must not have another
    chain's ready work queued behind it (head-of-line blocking).
  - (o, f) gates sit at partition base 0 and (i, j) at base 64 because
    walrus requires equal SBUF base partitions for 2-input DVE ops.
  - h is DMA'd out each step (SWDGE/HWDGE split); decode runs on host.
"""

import os
import sys

for _p in ("/opt/trn_rl_repo", "/root/.axon_site/_ro/trn_rl_repo"):
    if os.path.isdir(_p) and _p not in sys.path:
        sys.path.insert(0, _p)

import numpy as np

from concourse import bass, mybir, tile
from concourse.bass_utils import run_bass_kernel_spmd

# Problem constants (hardcoded per harness contract).
N, T, OBS, ACT, H = 4096, 256, 32, 8, 64
D = OBS + ACT          # 40
DX = D + 1             # x rows incl. the constant-1 bias row
FORGET_BIAS = 1.0
NCORES = 8
NB = N // NCORES       # 512 sequences per core
SZS = [172, 170, 170]  # chain batch sizes (sum = NB)
NCH = len(SZS)
OFFS = [sum(SZS[:i]) for i in range(NCH + 1)]
XCHUNK = 4             # timesteps per X window DMA
NFILL = 0              # PE p-state filler matmuls per step

AFT = mybir.ActivationFunctionType
ALU = mybir.AluOpType
BF16 = mybir.dt.bfloat16
F32 = mybir.dt.float32

_BF16_NP = mybir.dt.np(BF16)


def _split_multi_waits(nc, max_waits=1):
    """Workaround for this walrus build's small per-instruction sync-wait
    capacity: hoist excess sem waits onto preceding same-engine NOPs.

    Engines execute in order, so a NOP carrying some of the waits right
    before the real instruction preserves semantics exactly.
    """
    def stale_first(w):
        nm = (w.ant_name or "")
        # DMA / PE / Pool sems are usually stale WAR edges; ACT/DVE sems
        # are usually the live RAW producer edge — keep those on the op.
        return 0 if nm.startswith(("DMA", "PE", "Pool", "SP")) else 1

    for f in nc.m.functions:
        for blk in f.blocks:
            out = []
            changed = False
            for inst in blk.instructions:
                si = inst.sync_info
                if si is not None and si.on_wait and len(si.on_wait) > max_waits:
                    waits = sorted(si.on_wait, key=stale_first)
                    extra, keep = waits[:-max_waits], waits[-max_waits:]
                    for i in range(0, len(extra), max_waits):
                        nop = mybir.InstNoOp(
                            name=f"{inst.name}-wsplit{i}",
                            ins=[],
                            outs=[],
                            engine=inst.engine,
                            sync_info=mybir.SyncInfo(
                                on_wait=extra[i:i + max_waits], on_update=[]
                            ),
                        )
                        out.append(nop)
                    inst.sync_info = mybir.SyncInfo(
                        on_wait=keep, on_update=list(si.on_update)
                    )
                    changed = True
                out.append(inst)
            if changed:
                blk.instructions = out


def _prep_weights(W_lstm, b_lstm):
    """Split/scale weights into (W1x, W1h, W2x, W2h).

    Gate pre-activations arranged so sigmoid(2*pre) is the right value:
    i, o, f columns halved (f gets +FORGET_BIAS folded), j kept as-is.
    The x-block rows are [W_x | bias]; the bias rides the constant-1 row.
    """
    W = np.asarray(W_lstm, np.float64)
    b = np.asarray(b_lstm, np.float64)
    W_x, W_h = W[:D], W[D:]
    cols = {k: slice(i * H, (i + 1) * H) for i, k in enumerate("ijfo")}

    def blocks(gate, scale, bias_add):
        wx = W_x[:, cols[gate]] * scale
        wh = W_h[:, cols[gate]] * scale
        bb = (b[cols[gate]] + bias_add) * scale
        return np.concatenate([wx, bb[None, :]], axis=0), wh  # [41,64],[64,64]

    xo, ho = blocks("o", 0.5, 0.0)
    xi, hi = blocks("i", 0.5, 0.0)
    xf, hf = blocks("f", 0.5, FORGET_BIAS)
    xj, hj = blocks("j", 1.0, 0.0)
    # Partition-base pairing: (o, f) at psum parts [0:64], (i, j) at [64:128].
    W1x = np.concatenate([xo, xi], axis=1)  # [41, 128]
    W1h = np.concatenate([ho, hi], axis=1)  # [64, 128]
    W2x = np.concatenate([xf, xj], axis=1)
    W2h = np.concatenate([hf, hj], axis=1)
    return W1x, W1h, W2x, W2h


def _build_nc():
    """Build the SPMD bass program (identical on all 8 cores)."""
    nc = bass.Bass()
    X = nc.declare_dram_parameter("x", [T, DX, NB], BF16, isOutput=False)
    W1xd = nc.declare_dram_parameter("w1x", [DX, 2 * H], BF16, isOutput=False)
    W1hd = nc.declare_dram_parameter("w1h", [H, 2 * H], BF16, isOutput=False)
    W2xd = nc.declare_dram_parameter("w2x", [DX, 2 * H], BF16, isOutput=False)
    W2hd = nc.declare_dram_parameter("w2h", [H, 2 * H], BF16, isOutput=False)
    HS = nc.declare_dram_parameter("hs_out", [T, H, NB], BF16, isOutput=True)

    with tile.TileContext(nc) as tc:
        with (
            tc.tile_pool(name="wpool", bufs=1) as wpool,
            tc.tile_pool(name="xw", bufs=6) as xwp,
            tc.tile_pool(name="hp", bufs=10) as hp,
            tc.tile_pool(name="ps", bufs=2, space="PSUM") as psp,
            tc.tile_pool(name="sig", bufs=12) as sigp,
            tc.tile_pool(name="small", bufs=18) as smallp,
            tc.tile_pool(name="cst", bufs=10) as cstp,
            tc.tile_pool(name="fill", bufs=2, space="PSUM") as fillp,
        ):
            w1x = wpool.tile([DX, 2 * H], BF16, tag="w1x")
            w1h = wpool.tile([H, 2 * H], BF16, tag="w1h")
            w2x = wpool.tile([DX, 2 * H], BF16, tag="w2x")
            w2h = wpool.tile([H, 2 * H], BF16, tag="w2h")
            nc.sync.dma_start(w1x[:], W1xd[:])
            nc.sync.dma_start(w1h[:], W1hd[:])
            nc.sync.dma_start(w2x[:], W2xd[:])
            nc.sync.dma_start(w2h[:], W2hd[:])

            # X windows: per chain, XCHUNK steps per tile, triple-buffered.
            xwin = [{} for _ in range(NCH)]

            def load_xwin(ch, k):
                cb = SZS[ch]
                csl = slice(OFFS[ch], OFFS[ch + 1])
                xt = xwp.tile(
                    [DX, XCHUNK * cb], BF16, tag=f"xw{ch}", name=f"xw{ch}_{k}"
                )
                nc.sync.dma_start(
                    xt[:],
                    X[k * XCHUNK:(k + 1) * XCHUNK, :, csl].rearrange(
                        "t f n -> f t n"
                    ),
                )
                xwin[ch][k] = xt

            for ch in range(NCH):
                load_xwin(ch, 0)
                load_xwin(ch, 1)

            h_cur = [None] * NCH
            c_cur = [None] * NCH
            for ch in range(NCH):
                cb = SZS[ch]
                h0 = hp.tile([H, cb], BF16, tag=f"h{ch}", name=f"h{ch}_init")
                nc.vector.memset(h0[:], 0.0)
                c0 = cstp.tile([H, cb], BF16, tag=f"c{ch}", name=f"c{ch}_init")
                nc.vector.memset(c0[:], 0.0)
                h_cur[ch] = h0
                c_cur[ch] = c0

            def xslice(ch, t):
                cb = SZS[ch]
                tl = t % XCHUNK
                return xwin[ch][t // XCHUNK][:, tl * cb:(tl + 1) * cb]

            for t in range(T):
                if t % XCHUNK == 0:
                    k = t // XCHUNK + 2  # prefetch the window after next
                    if k < T // XCHUNK:
                        for ch in range(NCH):
                            load_xwin(ch, k)

                # Phase-grouped emission across chains (see docstring).
                pss, ss, tjs, qs, ps_, cns, tcs = ({} for _ in range(7))

                def emit_h(ch):
                    cb = SZS[ch]
                    h_new = hp.tile(
                        [H, cb], BF16, tag=f"h{ch}", name=f"h{ch}_{t}"
                    )
                    nc.vector.tensor_mul(
                        h_new[:], tcs[ch], ss[ch][0:H, 0:cb]
                    )
                    h_cur[ch] = h_new

                def emit_cell(ch):
                    cb = SZS[ch]
                    tj = smallp.tile(
                        [2 * H, cb], BF16, tag=f"tj{ch}", name=f"tj{ch}_{t}"
                    )
                    tjs[ch] = tj
                    nc.vector.tensor_scalar(
                        tj[H:2 * H, :], ss[ch][H:2 * H, cb:2 * cb],
                        2.0, -1.0, ALU.mult, ALU.add,
                    )
                    q = smallp.tile(
                        [H, cb], BF16, tag=f"q{ch}", name=f"q{ch}_{t}"
                    )
                    qs[ch] = q
                    nc.vector.tensor_mul(
                        q[:], c_cur[ch][:], ss[ch][0:H, cb:2 * cb]
                    )
                    p = smallp.tile(
                        [H, cb], BF16, tag=f"p{ch}", name=f"p{ch}_{t}"
                    )
                    ps_[ch] = p
                    nc.vector.tensor_mul(
                        p[:], tjs[ch][H:2 * H, :], ss[ch][H:2 * H, 0:cb]
                    )
                    c_new = cstp.tile(
                        [H, cb], BF16, tag=f"c{ch}", name=f"c{ch}_{t}"
                    )
                    cns[ch] = c_new
                    nc.vector.tensor_add(c_new[:], ps_[ch][:], qs[ch][:])
                    c_cur[ch] = c_new
                    tc_t = smallp.tile(
                        [H, cb], BF16, tag=f"tc{ch}", name=f"tc{ch}_{t}"
                    )
                    tcs[ch] = tc_t
                    nc.scalar.activation(tc_t[:], c_new[:], AFT.Tanh)
                # PSUM accumulation groups within one tile must be
                # consecutive (interleaving them corrupts the bank), so the
                # per-tile matmul order is x2; h2; x1; h1 — only x2 can fire
                # early, the rest follow the h dependency.
                for ch in range(NCH):
                    cb = SZS[ch]
                    ps = psp.tile(
                        [2 * H, 2 * cb], F32, tag=f"ps{ch}", name=f"ps{ch}_{t}"
                    )
                    pss[ch] = ps
                    nc.tensor.matmul(
                        ps[:, cb:2 * cb], w2x[:], xslice(ch, t),
                        start=True, stop=False,
                    )
                for ch in range(NCH):
                    cb = SZS[ch]
                    nc.tensor.matmul(
                        pss[ch][:, cb:2 * cb], w2h[:], h_cur[ch][:],
                        start=False, stop=True,
                    )
                    nc.tensor.matmul(
                        pss[ch][:, 0:cb], w1x[:], xslice(ch, t),
                        start=True, stop=False,
                    )
                    nc.tensor.matmul(
                        pss[ch][:, 0:cb], w1h[:], h_cur[ch][:],
                        start=False, stop=True,
                    )
                # PE p-state fillers: keep the tensor engine continuously
                # busy so the cost of the critical h-matmuls stays at the
                # full-clock rate (pe ramp needs ~3us of continuous work).
                for fi in range(NFILL):
                    fl = fillp.tile(
                        [2 * H, 512], F32, tag="fill", name=f"fill{t}_{fi}"
                    )
                    nc.tensor.matmul(
                        fl[:], w1x[:], xwin[0][t // XCHUNK][:, 0:512],
                        start=True, stop=True,
                    )
                for ch in range(NCH):
                    cb = SZS[ch]
                    # S: parts [0:64] = (sig_o | sig_f1), [64:128] = (sig_i | sig_2j)
                    s = sigp.tile(
                        [2 * H, 2 * cb], BF16, tag=f"s{ch}", name=f"s{ch}_{t}"
                    )
                    ss[ch] = s
                    nc.scalar.activation(
                        s[:], pss[ch][:], AFT.Sigmoid, scale=2.0
                    )
                for ch in range(NCH):
                    emit_cell(ch)
                    if ch >= 1:
                        emit_h(ch - 1)
                emit_h(NCH - 1)
                for ch in range(NCH):
                    csl = slice(OFFS[ch], OFFS[ch + 1])
                    # Output DMAs split across the two descriptor generators
                    # (SWDGE on Pool, HWDGE on SP) so neither serializes.
                    if ch == 0:
                        nc.gpsimd.dma_start(HS[t, :, csl], h_cur[ch][:])
                    else:
                        nc.sync.dma_start(HS[t, :, csl], h_cur[ch][:])

    _split_multi_waits(nc)
    return nc


_NC_CACHE = None


def _get_nc():
    global _NC_CACHE
    if _NC_CACHE is None:
        _NC_CACHE = _build_nc()
    return _NC_CACHE


def kernel(obss, actions, W_lstm, b_lstm, W_dec, b_dec, _trace=False):
    obss = np.asarray(obss, np.float32)
    actions = np.asarray(actions, np.float32)

    # Host prep: x = [obs | act | 1] in feature-major per-core layout.
    x = np.concatenate(
        [obss, actions, np.ones((N, T, 1), np.float32)], axis=-1
    )  # [N, T, 41]
    W1x, W1h, W2x, W2h = _prep_weights(W_lstm, b_lstm)
    wmaps = {
        "w1x": W1x.astype(_BF16_NP),
        "w1h": W1h.astype(_BF16_NP),
        "w2x": W2x.astype(_BF16_NP),
        "w2h": W2h.astype(_BF16_NP),
    }

    in_maps = []
    for c in range(NCORES):
        xc = np.ascontiguousarray(
            x[c * NB:(c + 1) * NB].transpose(1, 2, 0)
        ).astype(_BF16_NP)  # [T, 41, NB]
        in_maps.append({"x": xc, **wmaps})

    nc = _get_nc()
    res = run_bass_kernel_spmd(nc, in_maps, list(range(NCORES)), trace=_trace)

    # Gather h shards [T, H, NB] -> [T, H, N]; decode on host.
    hs = np.concatenate(
        [res.results[c]["hs_out"].astype(np.float32) for c in range(NCORES)],
        axis=2,
    )
    wd = np.asarray(W_dec, np.float32)[:, 0]
    out = np.einsum("tfn,f->tn", hs, wd) + np.float32(
        np.asarray(b_dec, np.float32)[0]
    )
    out = out[:, :, None].astype(np.float32)  # [T, N, 1]
    if _trace:
        kernel.last_results = res
    return out

